# revision 66
# baseline (speedup 1.0000x reference)
"""Trainium2 Bass kernel for relative-position multi-head attention.

Problem shape (hardcoded): B=2, T=1024, CH=1024, HEADS=16, KC=64, WIN=4.
Sharding: tensor-parallel over heads across 8 cores (2 heads/core).  Each
core computes its 2 heads' attention output O_c and the PARTIAL output
projection P_c = O_c @ Wo[rows_c]; the host sums the 8 partials (+bo), so
no device collective is needed.

The relative-position band (9 diagonals |j - t| <= 4 of the [T, T] score
matrix) is handled entirely on-chip, with no DRAM staging:
  * banded raw scores s9[m, t] = k_{t+m-4} . q_t are computed directly as
    shifted elementwise products of kT/qT (free-axis slices are free in
    SBUF) reduced over d via ones-vector matmuls;
  * the band bias R[m, t] = erk[:, m] . q_t comes from a tiny matmul;
  * g9 = exp(s9 + R) feeds the rel-v epilogue (erv matmul), and
    delta = g9 * (1 - exp(-R)) = exp(s9+R) - exp(s9) corrects both the
    PV numerator (via 9 shifted broadcast-multiply-adds against vT) and
    the softmax normalizer (ones9 matmul into the colsum row), so the
    main P*V matmul can use plain exp(scores) with no band add at all.
"""

import sys

sys.path.insert(0, "/opt/trn_rl_repo")

import math
import numpy as np

import concourse.bass as bass
import concourse.tile as tile
from concourse import mybir
from concourse import bacc
from concourse.bass_utils import run_bass_kernel_spmd
from concourse.masks import make_identity

# ---------------------------------------------------------------- constants
B, T, CH, HEADS, KC, WIN = 2, 1024, 1024, 16, 64, 4
NCORES = 8
HPC = HEADS // NCORES          # heads per core = 2
DPC = HPC * KC                 # channels per core = 128
NI = B * T                     # 2048 flattened (b, t)
M9 = 2 * WIN + 1               # 9 diagonals
F32 = mybir.dt.float32
F32R = mybir.dt.float32r
BF16 = mybir.dt.bfloat16
F16 = mybir.dt.float16
AF = mybir.ActivationFunctionType
ALU = mybir.AluOpType

_CACHE = {}


# ---------------------------------------------------------------- program
def build_program():
    nc = bacc.Bacc("TRN2", target_bir_lowering=False, debug=False,
                   num_devices=NCORES)

    xT = nc.dram_tensor("xT", [CH, NI], BF16, kind="ExternalInput")
    cT = nc.dram_tensor("cT", [CH, NI], BF16, kind="ExternalInput")
    # wq/wk/wv host-prechunked: [128, 8*DPC], chunk d8 = rows 128*d8..+127
    wq = nc.dram_tensor("wq", [128, 8 * DPC], BF16, kind="ExternalInput")
    wk = nc.dram_tensor("wk", [128, 8 * DPC], BF16, kind="ExternalInput")
    wv = nc.dram_tensor("wv", [128, 8 * DPC], BF16, kind="ExternalInput")
    # wo = Wo[core_rows, :]: [DPC, CH]
    wo = nc.dram_tensor("wo", [DPC, CH], F32R, kind="ExternalInput")
    bq = nc.dram_tensor("bq", [DPC, 1], F32, kind="ExternalInput")
    bk = nc.dram_tensor("bk", [DPC, 1], F32, kind="ExternalInput")
    bv = nc.dram_tensor("bv", [DPC, 1], F32, kind="ExternalInput")
    erk = nc.dram_tensor("erk", [DPC, M9], F32R, kind="ExternalInput")
    erv = nc.dram_tensor("erv", [M9, KC + 1], F32R, kind="ExternalInput")
    e9c = nc.dram_tensor("e9c", [KC, 9 * M9], BF16, kind="ExternalInput")
    # partial projection output P_c^T blocks: po[p, o8*NI + n]
    po = nc.dram_tensor("po", [128, 8 * NI], F16, kind="ExternalOutput")

    with tile.TileContext(nc) as tc:
        with (
            tc.tile_pool(name="const", bufs=1) as cpool,
            tc.tile_pool(name="persist", bufs=1) as ppool,
        ):
            # ---------------- constants / weights to SBUF (1 DMA each)
            wsb = {}
            for nm, src in (("wq", wq), ("wk", wk), ("wv", wv)):
                t_ = cpool.tile([128, 8 * DPC], BF16, name=f"{nm}_sb")
                nc.scalar.dma_start(t_[:], src[:])
                wsb[nm] = t_

            def wtile(nm, d8):
                return wsb[nm][:, DPC * d8:DPC * (d8 + 1)]

            wo_sb = cpool.tile([DPC, CH], F32R, name="wo_sb")
            nc.scalar.dma_start(wo_sb[:], wo[:])
            bq_sb = cpool.tile([DPC, 1], F32, name="bq_sb")
            bk_sb = cpool.tile([DPC, 1], F32, name="bk_sb")
            bv_sb = cpool.tile([DPC, 1], F32, name="bv_sb")
            for t_, src in ((bq_sb, bq), (bk_sb, bk), (bv_sb, bv)):
                nc.scalar.dma_start(t_[:], src[:])
            erk_sb = cpool.tile([DPC, M9], F32R, name="erk_sb")
            nc.scalar.dma_start(erk_sb[:], erk[:])
            erv_sb = cpool.tile([M9, KC + 1], F32R, name="erv_sb")
            nc.scalar.dma_start(erv_sb[:], erv[:])
            ident = cpool.tile([128, 128], F32, name="ident")
            make_identity(nc, ident[:])
            idb = cpool.tile([128, 128], BF16, name="idb")
            nc.scalar.activation(idb[:], ident[:], AF.Copy)
            ones8 = cpool.tile([128, 8], F32, name="ones8")
            nc.vector.memset(ones8[:], 1.0)
            # e9lhs[:, 9m:9m+9] is a [KC, 9] lhsT whose column m is all-ones:
            # matmul with it lands sum_d(rhs) in PSUM row m, zeros elsewhere
            e9lhs = cpool.tile([KC, 9 * M9], BF16, name="e9lhs")
            nc.scalar.dma_start(e9lhs[:], e9c[:])

            # persistent activations
            qT_sb = ppool.tile([DPC, NI], F32R, name="qT_sb")
            kT_sb = ppool.tile([DPC, NI], F32R, name="kT_sb")
            vT_sb = ppool.tile([DPC, NI], BF16, name="vT_sb")
            OT_sb = ppool.tile([DPC, NI], F32R, name="OT_sb")
            # v in [j, d] layout + ones column, per unit (b, h): [128, 8*65]
            vju = [ppool.tile([128, 8 * (KC + 1)], F32R, name=f"vju{u}")
                   for u in range(4)]
            # ones columns of vju (one strided SBUF->SBUF DMA per unit)
            for u in range(4):
                dst = bass.AP(vju[u].tensor, KC,
                              [[8 * (KC + 1), 128], [KC + 1, 8]])
                nc.scalar.dma_start(dst, ones8[:].bitcast(F32R))

            # ---------------- phase A: QKV projections (transposed layouts)
            with (
                tc.tile_pool(name="xin", bufs=10) as xpool,
                tc.tile_pool(name="qkvps", bufs=1, space="PSUM") as qkvps,
                tc.tile_pool(name="tps", bufs=2, space="PSUM") as tpps,
            ):
                xts = []
                for d8 in range(8):
                    t_ = xpool.tile([128, NI], BF16, tag="xt")
                    nc.gpsimd.dma_start(t_[:], xT[d8 * 128:(d8 + 1) * 128, :])
                    xts.append(t_)
                cts = []
                for d8 in range(8):
                    t_ = xpool.tile([128, NI], BF16, tag="xt")
                    nc.sync.dma_start(t_[:], cT[d8 * 128:(d8 + 1) * 128, :])
                    cts.append(t_)
                qps = [qkvps.tile([DPC, 512], F32, tag=f"proj{it}",
                                  name=f"qp{it}") for it in range(4)]
                for d8 in range(8):
                    for it in range(4):
                        nc.tensor.matmul(
                            qps[it][:], wtile("wq", d8),
                            xts[d8][:, it * 512:(it + 1) * 512],
                            start=(d8 == 0), stop=(d8 == 7))
                for it in range(4):
                    nc.vector.tensor_scalar_add(
                        qT_sb[:, it * 512:(it + 1) * 512], qps[it][:], bq_sb[:])

                kps = [qkvps.tile([DPC, 512], F32, tag=f"proj{it}",
                                  name=f"kp{it}") for it in range(4)]
                for d8 in range(8):
                    for it in range(4):
                        nc.tensor.matmul(
                            kps[it][:], wtile("wk", d8),
                            cts[d8][:, it * 512:(it + 1) * 512],
                            start=(d8 == 0), stop=(d8 == 7))
                for it in range(4):
                    nc.scalar.activation(kT_sb[:, it * 512:(it + 1) * 512],
                                         kps[it][:], AF.Identity, bias=bk_sb[:])
                vps = [qkvps.tile([DPC, 512], F32, tag=f"proj{it}",
                                  name=f"vp{it}") for it in range(4)]
                for d8 in range(8):
                    for it in range(4):
                        nc.tensor.matmul(
                            vps[it][:], wtile("wv", d8),
                            cts[d8][:, it * 512:(it + 1) * 512],
                            start=(d8 == 0), stop=(d8 == 7))
                for it in range(4):
                    nc.vector.tensor_scalar_add(
                        vT_sb[:, it * 512:(it + 1) * 512], vps[it][:], bv_sb[:])

                # transpose v to [j, d] per unit; ones col already DMA'd
                for u in range(4):
                    b, h = divmod(u, 2)
                    for jc in range(8):
                        tp = tpps.tile([128, KC], BF16, tag="tp")
                        nc.tensor.transpose(
                            tp[:],
                            vT_sb[64 * h:64 * h + 64,
                                  1024 * b + 128 * jc:1024 * b + 128 * (jc + 1)],
                            idb[64 * h:64 * h + 64, 64 * h:64 * h + 64])
                        nc.scalar.activation(
                            vju[u][:, 65 * jc:65 * jc + 64], tp[:], AF.Copy)

            # ---------------- phase B: attention per unit
            with (
                tc.tile_pool(name="rpp", bufs=1, space="PSUM") as rppool,
                tc.tile_pool(name="spool", bufs=3, space="PSUM") as spool,
                tc.tile_pool(name="opool", bufs=1, space="PSUM") as opool,
                tc.tile_pool(name="tmp", bufs=4) as tpool,
                tc.tile_pool(name="exps", bufs=3) as exppool,
                tc.tile_pool(name="nine", bufs=12) as npool,
                tc.tile_pool(name="bcast", bufs=2) as bpool,
                tc.tile_pool(name="accp", bufs=2) as apool,
                tc.tile_pool(name="misc", bufs=4) as mpool,
                tc.tile_pool(name="dram", bufs=2, space="DRAM") as drpool,
            ):
                for u in range(4):
                    b, h = divmod(u, 2)
                    hb = 64 * h
                    ib = 1024 * b

                    # --- banded-diagonal pipeline (all [9, T])
                    # R[m, t] = erk[:, m] . q_t (chunked, copied to SBUF)
                    rp_sb = npool.tile([M9, T], F32, tag="n9")
                    for s in range(2):
                        rpc = rppool.tile([M9, 512], F32, tag="rp",
                                          name=f"rp{u}_{s}")
                        nc.tensor.matmul(
                            rpc[:],
                            erk_sb[hb:hb + 64, :],
                            qT_sb[hb:hb + 64, ib + 512 * s:ib + 512 * (s + 1)],
                            start=True, stop=True)
                        nc.scalar.activation(rp_sb[:, 512 * s:512 * (s + 1)],
                                             rpc[:], AF.Copy)
                    # s9[m, t] = k_{t+m-4} . q_t via shifted products (DVE),
                    # d-reduced on PE: one-hot-column lhsT lands each row sum
                    # in PSUM row m (zero-weighted adds elsewhere).  t-edges
                    # where j = t+m-4 is out of range are memset to -1e9 in
                    # tmp so exp(s9+R) and delta come out 0 there.
                    s9p = rppool.tile([M9, T], F32, tag="s9p",
                                      name=f"s9p{u}")
                    for m in range(M9):
                        o = m - WIN
                        t0 = max(0, -o)
                        t1 = T - max(0, o)
                        tmp = tpool.tile([KC, T], BF16, tag="tmp")
                        if t0 > 0:
                            nc.vector.memset(tmp[:, 0:t0], -1e9)
                        if t1 < T:
                            nc.vector.memset(tmp[:, t1:T], -1e9)
                        nc.vector.tensor_mul(
                            tmp[:, t0:t1],
                            kT_sb[hb:hb + 64,
                                  ib + t0 + o:ib + t1 + o].bitcast(F32),
                            qT_sb[hb:hb + 64, ib + t0:ib + t1].bitcast(F32))
                        for s in range(2):
                            nc.tensor.matmul(
                                s9p[:, 512 * s:512 * (s + 1)],
                                e9lhs[:, 9 * m:9 * (m + 1)],
                                tmp[:, 512 * s:512 * (s + 1)],
                                start=(m == 0), stop=(m == M9 - 1))
                    # g9 = exp(s9 + R); delta = g9 * (1 - exp(-R))
                    bsum9 = npool.tile([M9, T], F32, tag="n9")
                    nc.vector.tensor_add(bsum9[:], s9p[:], rp_sb[:])
                    g9sb = npool.tile([M9, T], F32R, tag="n9")
                    nc.scalar.activation(g9sb[:], bsum9[:], AF.Exp)
                    enr = npool.tile([M9, T], F32, tag="n9")
                    nc.scalar.activation(enr[:], rp_sb[:], AF.Exp, scale=-1.0)
                    w9 = npool.tile([M9, T], F32, tag="n9")
                    nc.vector.tensor_scalar(w9[:], enr[:], -1.0, 1.0,
                                            ALU.mult, ALU.add)
                    d9sb = npool.tile([M9, T], BF16, tag="n9")
                    nc.vector.tensor_mul(d9sb[:], g9sb[:].bitcast(F32), w9[:])

                    # --- PV-numerator band correction:
                    # acc[d, t] = sum_m delta[m, t] * v[t+m-4, d]
                    # delta rows are partition-broadcast by DMA via a DRAM
                    # bounce: 64 stride-0 reads of row m replicate it across
                    # partitions hb..hb+63 (SBUF APs forbid stride-0 partition
                    # dims, DRAM APs don't).
                    d9d = drpool.tile([M9, T], BF16, name=f"d9d{u}",
                                      tag="d9d")
                    nc.sync.dma_start(d9d[:], d9sb[:])

                    def bcast_row(dst_ap, row, eng):
                        src = bass.AP(d9d.tensor, row * T, [[0, 64], [1, T]])
                        eng.dma_start(dst_ap, src)

                    engs = [nc.sync, nc.scalar, nc.gpsimd]
                    acc = apool.tile([KC, T], BF16, tag="acc")
                    db4 = bpool.tile([128, T], BF16, tag="db")
                    bcast_row(db4[hb:hb + 64, :], WIN, engs[0])
                    nc.vector.tensor_mul(
                        acc[:], vT_sb[hb:hb + 64, ib:ib + T],
                        db4[hb:hb + 64, :])
                    for m in range(M9):
                        o = m - WIN
                        if o == 0:
                            continue
                        t0 = max(0, -o)
                        t1 = T - max(0, o)
                        db = bpool.tile([128, T], BF16, tag="db")
                        bcast_row(db[hb:hb + 64, :], m, engs[m % 3])
                        tmp2 = tpool.tile([KC, T], BF16, tag="tmp2")
                        nc.vector.tensor_mul(
                            tmp2[:, t0:t1],
                            vT_sb[hb:hb + 64, ib + t0 + o:ib + t1 + o],
                            db[hb:hb + 64, t0:t1])
                        nc.vector.tensor_add(acc[:, t0:t1], acc[:, t0:t1],
                                             tmp2[:, t0:t1])

                    # --- main attention loop
                    op = [opool.tile([KC + 1, 512], F32, tag=f"ops{s}",
                                     name=f"ops{s}_{u}") for s in range(2)]
                    for jt in range(8):
                        j0 = 128 * jt
                        for s in range(2):
                            sp = spool.tile([128, 512], F32, tag="sps")
                            nc.tensor.matmul(
                                sp[:],
                                kT_sb[hb:hb + 64, ib + j0:ib + j0 + 128],
                                qT_sb[hb:hb + 64,
                                      ib + 512 * s:ib + 512 * (s + 1)],
                                start=True, stop=True)
                            expt = exppool.tile([128, 512], F32R, tag="expt")
                            nc.scalar.activation(expt[:], sp[:], AF.Exp)
                            nc.tensor.matmul(
                                op[s][:],
                                vju[u][:, 65 * jt:65 * (jt + 1)],
                                expt[:],
                                start=(jt == 0), stop=False)
                    # rel-v epilogue
                    for s in range(2):
                        nc.tensor.matmul(
                            op[s][:], erv_sb[:],
                            g9sb[:, 512 * s:512 * (s + 1)],
                            start=False, stop=True)
                    # colsum correction: dsum[t] = sum_m delta[m, t]
                    dsum = npool.tile([M9, T], F32, tag="n9")
                    nc.gpsimd.partition_all_reduce(
                        dsum[:], d9sb[:], M9, bass.bass_isa.ReduceOp.add)

                    # --- normalize: OT = (op[0:KC] + acc) / (colsum + dsum)
                    cs1 = mpool.tile([1, T], F32, tag="cs1")
                    for s in range(2):
                        nc.scalar.activation(cs1[:, 512 * s:512 * (s + 1)],
                                             op[s][KC:KC + 1, :], AF.Copy)
                    nc.vector.tensor_add(cs1[:], cs1[:], dsum[0:1, :])
                    rcp1 = mpool.tile([1, T], F32, tag="rcp1")
                    nc.vector.reciprocal_approx_fast(rcp1[:], cs1[:])
                    accf = apool.tile([KC, T], F32, tag="accf")
                    nc.scalar.activation(accf[:], acc[:], AF.Copy)
                    rcpb = bpool.tile([KC, T], F32, tag="rb")
                    nc.gpsimd.partition_broadcast(rcpb[:], rcp1[:])
                    for s in range(2):
                        sl = slice(512 * s, 512 * (s + 1))
                        t3 = tpool.tile([KC, T], F32, tag="tmp")
                        nc.vector.tensor_add(t3[:, sl], op[s][0:KC, :],
                                             accf[:, sl])
                        nc.vector.tensor_mul(
                            OT_sb[hb:hb + 64, ib + 512 * s:ib + 512 * (s + 1)],
                            t3[:, sl], rcpb[:, sl])

            # ---------------- phase C: partial output projection (no AllGather)
            with (
                tc.tile_pool(name="fps", bufs=4, space="PSUM") as fpool,
                tc.tile_pool(name="osb", bufs=4) as opool2,
            ):
                qnames = [nc.sync, nc.scalar, nc.gpsimd, nc.sync]
                for o8 in range(8):
                    for s4 in range(4):
                        fp = fpool.tile([128, 512], F32, tag="fps")
                        nc.tensor.matmul(
                            fp[:], wo_sb[:, 128 * o8:128 * (o8 + 1)],
                            OT_sb[:, 512 * s4:512 * (s4 + 1)],
                            start=True, stop=True)
                        ot = opool2.tile([128, 512], F16, tag="osb")
                        if s4 % 2:
                            nc.scalar.activation(ot[:], fp[:], AF.Copy)
                        else:
                            nc.vector.tensor_copy(ot[:], fp[:])
                        qnames[s4].dma_start(
                            po[:, NI * o8 + 512 * s4:NI * o8 + 512 * (s4 + 1)],
                            ot[:])

    nc.compile()
    return nc


# ---------------------------------------------------------------- host side
def _prep_inputs(x, c, Wq, bq, Wk, bk, Wv, bv, Wo, bo, emb_rel_k, emb_rel_v):
    import ml_dtypes
    bf16 = ml_dtypes.bfloat16
    scale = 1.0 / math.sqrt(KC)
    xT = np.ascontiguousarray(x.reshape(NI, CH).T.astype(bf16))  # [CH, NI]
    cT = np.ascontiguousarray(c.reshape(NI, CH).T.astype(bf16))
    Wq_s = (Wq * scale).astype(np.float32)
    bq_s = (bq * scale).astype(np.float32)
    erv_p = np.concatenate(
        [emb_rel_v[0], np.zeros((M9, 1), np.float32)], axis=1)  # [9, 65]
    erk2 = np.ascontiguousarray(
        np.concatenate([emb_rel_k[0].T, emb_rel_k[0].T], axis=0))  # [128, 9]
    e9c = np.zeros((KC, 9 * M9), np.float32)
    for m in range(M9):
        e9c[:, 9 * m + m] = 1.0

    def chunk8(W):  # [CH, DPC] -> [128, 8*DPC] (chunk d8 = rows 128*d8..+127)
        return np.ascontiguousarray(
            W.reshape(8, 128, DPC).transpose(1, 0, 2).reshape(128, 8 * DPC))

    in_maps = []
    for cix in range(NCORES):
        sl = slice(cix * DPC, (cix + 1) * DPC)
        in_maps.append({
            "xT": xT, "cT": cT,
            "wq": chunk8(Wq_s[:, sl].astype(np.float32)).astype(bf16),
            "wk": chunk8(Wk[:, sl].astype(np.float32)).astype(bf16),
            "wv": chunk8(Wv[:, sl].astype(np.float32)).astype(bf16),
            "wo": np.ascontiguousarray(Wo[sl, :].astype(np.float32)),
            "bq": np.ascontiguousarray(bq_s[sl, None]),
            "bk": np.ascontiguousarray(bk[sl, None].astype(np.float32)),
            "bv": np.ascontiguousarray(bv[sl, None].astype(np.float32)),
            "erk": erk2.astype(np.float32),
            "erv": erv_p.astype(np.float32),
            "e9c": e9c.astype(bf16),
        })
    return in_maps


def _numpy_fallback(x, c, mask, Wq, bq, Wk, bk, Wv, bv, Wo, bo,
                    emb_rel_k, emb_rel_v):
    # general-mask reference path (never taken for the spec'd all-ones mask)
    q = (x.reshape(NI, CH) @ Wq + bq).reshape(B, T, HEADS, KC).transpose(0, 2, 1, 3)
    k = (c.reshape(NI, CH) @ Wk + bk).reshape(B, T, HEADS, KC).transpose(0, 2, 1, 3)
    v = (c.reshape(NI, CH) @ Wv + bv).reshape(B, T, HEADS, KC).transpose(0, 2, 1, 3)
    qs = q / math.sqrt(KC)
    scores = np.einsum("bhtd,bhsd->bhts", qs, k)
    idx_j = np.arange(T)[None, :] - np.arange(T)[:, None] + WIN  # j - i + 4
    band = (idx_j >= 0) & (idx_j <= 2 * WIN)
    rel = np.einsum("bhtd,md->bhtm", qs, emb_rel_k[0])  # [B,H,T,9]
    bias = np.zeros((B, HEADS, T, T), np.float32)
    ii, jj = np.nonzero(band)
    bias[:, :, ii, jj] = rel[:, :, ii, idx_j[ii, jj]]
    scores = scores + bias
    scores = np.where(mask == 0, np.float32(1e-4), scores)
    scores -= scores.max(axis=-1, keepdims=True)
    p = np.exp(scores)
    p /= p.sum(axis=-1, keepdims=True)
    out = np.einsum("bhts,bhsd->bhtd", p, v)
    relw = np.zeros((B, HEADS, T, M9), np.float32)
    relw[:, :, ii, idx_j[ii, jj]] = p[:, :, ii, jj]
    out = out + np.einsum("bhtm,md->bhtd", relw, emb_rel_v[0])
    out = out.transpose(0, 2, 1, 3).reshape(NI, CH)
    return (out @ Wo + bo).reshape(B, T, CH).astype(np.float32)


def kernel(x, c, mask, Wq, bq, Wk, bk, Wv, bv, Wo, bo, emb_rel_k, emb_rel_v,
           _collect=None):
    x = np.asarray(x); c = np.asarray(c); mask = np.asarray(mask)
    args = [np.asarray(a) for a in
            (Wq, bq, Wk, bk, Wv, bv, Wo, bo, emb_rel_k, emb_rel_v)]
    if not np.all(mask):
        return _numpy_fallback(x, c, mask, *args)

    if "nc" not in _CACHE:
        _CACHE["nc"] = build_program()
    nc = _CACHE["nc"]

    in_maps = _prep_inputs(x, c, *args)
    res = run_bass_kernel_spmd(nc, in_maps, core_ids=list(range(NCORES)))
    if _collect is not None:
        _collect.append(res)
    # sum the 8 partial projections, reassemble [CH, NI], add bo
    acc = res.results[0]["po"].astype(np.float32)
    for cix in range(1, NCORES):
        acc = acc + res.results[cix]["po"].astype(np.float32)
    outT = acc.reshape(128, 8, NI).transpose(1, 0, 2).reshape(CH, NI)
    out = outT.T + np.asarray(args[7]).astype(np.float32)[None, :]
    return np.ascontiguousarray(out).reshape(B, T, CH).astype(np.float32)
